# revision 3
# baseline (speedup 1.0000x reference)
"""PointNet++ MSG set-abstraction kernel, sharded over 8 NeuronCores.

Strategy (per sharding hint): data-parallel over the 16 pieces, 2 pieces per
core. kNN + gathers + MLP + BN + maxpool run on the neuron cores; BatchNorm
(training stats) uses psum collectives across the 8 cores. FPS is a
1023-step sequential dependence chain (argmax -> distance update); XLA
`while` loops don't compile on this neuron stack, so it runs as a jitted
XLA-CPU scan on host over all 16 pieces.
"""
import numpy as np
import jax
import jax.numpy as jnp
from jax import lax
from functools import partial

N = 65536
P = 16
NP = N // P          # 4096 points per piece
SP = NP // 4         # 1024 centroids per piece
S = P * SP           # 16384 centroids
K_LIST = [32, 64]
IN_CH = 9
EPS = 1e-5
NDEV = 8
PPD = P // NDEV      # pieces per device = 2


def _fps_one(pts):
    """Farthest point sampling within one piece. pts: [NP, 3] -> [SP] local idx."""
    iota = jnp.arange(NP, dtype=jnp.int32)

    def body(carry, _):
        dist, last = carry
        d = jnp.sum((pts - pts[last]) ** 2, axis=-1)
        dist = jnp.minimum(dist, d)
        m = jnp.max(dist)
        nxt = jnp.min(jnp.where(dist == m, iota, NP)).astype(jnp.int32)
        return (dist, nxt), nxt

    init = (jnp.full((NP,), jnp.inf, pts.dtype), jnp.zeros((), jnp.int32))
    (_, _), idx = lax.scan(body, init, None, length=SP - 1)
    return jnp.concatenate([jnp.zeros((1,), jnp.int32), idx])


_fps_all_cpu = None


def _get_fps_all():
    global _fps_all_cpu
    if _fps_all_cpu is None:
        _fps_all_cpu = jax.jit(jax.vmap(_fps_one), backend="cpu")
    return _fps_all_cpu


@partial(jax.pmap, axis_name='d')
def _fwd(xp, feat, new_y, params0, params1):
    # xp: [PPD, NP, 3], feat: [PPD, NP, 9], new_y: [PPD, SP, 3]
    outs = []
    for K, params in zip(K_LIST, (params0, params1)):
        d2 = jnp.sum((new_y[:, :, None, :] - xp[:, None, :, :]) ** 2, -1)
        idx = lax.top_k(-d2, K)[1]                               # [PPD, SP, K]
        gxyz = jax.vmap(lambda a, i: a[i])(xp, idx)              # [PPD,SP,K,3]
        gfeat = jax.vmap(lambda a, i: a[i])(feat, idx)           # [PPD,SP,K,9]
        gx = gxyz - new_y[:, :, None, :]
        h = jnp.concatenate([gfeat, gx], axis=-1).reshape(PPD * SP, K, 12)
        cnt = float(S * K)
        for (W, b, g, be) in params:
            h = jnp.einsum('skc,oc->sko', h, W) + b
            sm = lax.psum(jnp.sum(h, axis=(0, 1)), 'd')
            sq = lax.psum(jnp.sum(h * h, axis=(0, 1)), 'd')
            mean = sm / cnt
            var = sq / cnt - mean * mean
            h = (h - mean) * lax.rsqrt(var + EPS) * g + be
            h = jax.nn.relu(h)
        outs.append(jnp.max(h, axis=1))
    return jnp.concatenate(outs, axis=-1)                        # [PPD*SP, 192]


def kernel(xyz, piece_id, points, params0, params1):
    xyz = np.asarray(xyz)
    piece_id_np = np.asarray(piece_id)
    points = np.asarray(points)

    x = np.ascontiguousarray(xyz[0].T).reshape(P, NP, 3)
    f = np.ascontiguousarray(points[0].T).reshape(P, NP, IN_CH)

    local = np.asarray(_get_fps_all()(x))                 # [P, SP] int32
    piece_base = (np.arange(P, dtype=np.int64) * NP)[:, None]
    centroids = (local.astype(np.int64) + piece_base).reshape(S)

    x_flat = x.reshape(N, 3)
    new_xyz_rows = x_flat[centroids]                      # [S, 3]
    new_y = new_xyz_rows.reshape(NDEV, PPD, SP, 3)

    xp = x.reshape(NDEV, PPD, NP, 3)
    fp = f.reshape(NDEV, PPD, NP, IN_CH)

    rep = lambda a: jnp.broadcast_to(jnp.asarray(a), (NDEV,) + np.shape(a))
    p0 = jax.tree_util.tree_map(rep, tuple(tuple(t) for t in params0))
    p1 = jax.tree_util.tree_map(rep, tuple(tuple(t) for t in params1))

    feats = _fwd(jnp.asarray(xp), jnp.asarray(fp), jnp.asarray(new_y), p0, p1)
    feats = np.asarray(feats)                             # [NDEV, PPD*SP, 192]

    new_xyz = new_xyz_rows.T[None].astype(np.float32)             # [1, 3, S]
    new_pid = piece_id_np[0, 0][centroids][None, None, :]          # [1, 1, S]
    new_points = feats.reshape(S, 192).T[None].astype(np.float32)  # [1, 192, S]
    return new_xyz, new_pid, new_points


# revision 7
# speedup vs baseline: 1.4000x; 1.4000x over previous
"""PointNet++ MSG set-abstraction kernel, sharded over 8 NeuronCores.

Strategy (per sharding hint): data-parallel over the 16 pieces, 2 pieces per
core. kNN + gathers + MLP + BN + maxpool run on the neuron cores; BatchNorm
(training stats) uses psum collectives across the 8 cores. FPS is a
1023-step sequential dependence chain (argmax -> distance update); XLA
`while` loops don't compile on this neuron stack, so it runs as a jitted
XLA-CPU scan on host over all 16 pieces.
"""
import os
# Parallelize the host-side FPS scan across CPU "devices" (threads). Must be
# set before the CPU backend initializes; harmless if already initialized.
os.environ.setdefault("XLA_FLAGS", "--xla_force_host_platform_device_count=16")

import numpy as np
import jax
import jax.numpy as jnp
from jax import lax
from functools import partial

N = 65536
P = 16
NP = N // P          # 4096 points per piece
SP = NP // 4         # 1024 centroids per piece
S = P * SP           # 16384 centroids
K_LIST = [32, 64]
IN_CH = 9
EPS = 1e-5
NDEV = 8
PPD = P // NDEV      # pieces per device = 2


def _fps_one(pts):
    """Farthest point sampling within one piece. pts: [NP, 3] -> [SP] local idx."""
    iota = jnp.arange(NP, dtype=jnp.int32)

    def body(carry, _):
        dist, last = carry
        d = jnp.sum((pts - pts[last]) ** 2, axis=-1)
        dist = jnp.minimum(dist, d)
        m = jnp.max(dist)
        nxt = jnp.min(jnp.where(dist == m, iota, NP)).astype(jnp.int32)
        return (dist, nxt), nxt

    init = (jnp.full((NP,), jnp.inf, pts.dtype), jnp.zeros((), jnp.int32))
    (_, _), idx = lax.scan(body, init, None, length=SP - 1)
    return jnp.concatenate([jnp.zeros((1,), jnp.int32), idx])


_fps_all_cpu = None


def _get_fps_all():
    """FPS over all 16 pieces on host: pmap across CPU host devices when
    available (one scan per thread), else a single jitted vmap."""
    global _fps_all_cpu
    if _fps_all_cpu is None:
        cpus = jax.devices("cpu")
        if len(cpus) >= P:
            inner = jax.pmap(_fps_one, devices=cpus[:P], backend="cpu")
            _fps_all_cpu = lambda x: inner(x)
        elif len(cpus) > 1:
            nd = max(d for d in range(1, len(cpus) + 1) if P % d == 0)
            inner = jax.pmap(jax.vmap(_fps_one), devices=cpus[:nd], backend="cpu")
            _fps_all_cpu = lambda x: np.asarray(
                inner(x.reshape(nd, P // nd, NP, 3))).reshape(P, SP)
        else:
            _fps_all_cpu = jax.jit(jax.vmap(_fps_one), backend="cpu")
    return _fps_all_cpu


@partial(jax.pmap, axis_name='d')
def _fwd(xp, feat, new_y, params0, params1):
    # xp: [PPD, NP, 3], feat: [PPD, NP, 9], new_y: [PPD, SP, 3]
    outs = []
    for K, params in zip(K_LIST, (params0, params1)):
        d2 = jnp.sum((new_y[:, :, None, :] - xp[:, None, :, :]) ** 2, -1)
        idx = lax.top_k(-d2, K)[1]                               # [PPD, SP, K]
        gxyz = jax.vmap(lambda a, i: a[i])(xp, idx)              # [PPD,SP,K,3]
        gfeat = jax.vmap(lambda a, i: a[i])(feat, idx)           # [PPD,SP,K,9]
        gx = gxyz - new_y[:, :, None, :]
        h = jnp.concatenate([gfeat, gx], axis=-1).reshape(PPD * SP, K, 12)
        cnt = float(S * K)
        for (W, b, g, be) in params:
            h = jnp.einsum('skc,oc->sko', h, W) + b
            sm = lax.psum(jnp.sum(h, axis=(0, 1)), 'd')
            sq = lax.psum(jnp.sum(h * h, axis=(0, 1)), 'd')
            mean = sm / cnt
            var = sq / cnt - mean * mean
            h = (h - mean) * lax.rsqrt(var + EPS) * g + be
            h = jax.nn.relu(h)
        outs.append(jnp.max(h, axis=1))
    return jnp.concatenate(outs, axis=-1)                        # [PPD*SP, 192]


def kernel(xyz, piece_id, points, params0, params1):
    xyz = np.asarray(xyz)
    piece_id_np = np.asarray(piece_id)
    points = np.asarray(points)

    x = np.ascontiguousarray(xyz[0].T).reshape(P, NP, 3)
    f = np.ascontiguousarray(points[0].T).reshape(P, NP, IN_CH)

    # Kick off async uploads of the piece data + params to the neuron cores,
    # then run FPS on host CPU while they transfer.
    xp = x.reshape(NDEV, PPD, NP, 3)
    fp = f.reshape(NDEV, PPD, NP, IN_CH)
    xp_d = jnp.asarray(xp)
    fp_d = jnp.asarray(fp)
    rep = lambda a: jnp.broadcast_to(jnp.asarray(a), (NDEV,) + np.shape(a))
    p0 = jax.tree_util.tree_map(rep, tuple(tuple(t) for t in params0))
    p1 = jax.tree_util.tree_map(rep, tuple(tuple(t) for t in params1))

    local = np.asarray(_get_fps_all()(x))                 # [P, SP] int32
    piece_base = (np.arange(P, dtype=np.int64) * NP)[:, None]
    centroids = (local.astype(np.int64) + piece_base).reshape(S)

    x_flat = x.reshape(N, 3)
    new_xyz_rows = x_flat[centroids]                      # [S, 3]
    new_y = new_xyz_rows.reshape(NDEV, PPD, SP, 3)

    feats = _fwd(xp_d, fp_d, jnp.asarray(new_y), p0, p1)
    feats = np.asarray(feats)                             # [NDEV, PPD*SP, 192]

    new_xyz = new_xyz_rows.T[None].astype(np.float32)             # [1, 3, S]
    new_pid = piece_id_np[0, 0][centroids][None, None, :]          # [1, 1, S]
    new_points = feats.reshape(S, 192).T[None].astype(np.float32)  # [1, 192, S]
    return new_xyz, new_pid, new_points
